# revision 19
# baseline (speedup 1.0000x reference)
"""Trainium2 Bass kernel for CommutatorConv2d.

Math: with lambda_c=0, lambda_a=1 the reference is a conv2d with effective
kernel  w_eff[o,i,r,s] = krow[o,i,s] + kcol[o,i,r]  (krow = sum_r w, kcol =
sum_s w).  That kernel lives in a 5-dim spatial-filter subspace
(span{1_r x e_s} + span{e_r x 1_s} has dim 3+3-1), so the 9-tap conv
factors into FIVE accumulating matmuls (contraction 128 each):

  y[o,h,w] = sum_{i,s} (krow[o,i,s]+kcol[o,i,1]) * xv[i, h, w+s-1]
           + sum_i (kcol[o,i,0]-kcol[o,i,1]) * xh[i, h-1, w]
           + sum_i (kcol[o,i,2]-kcol[o,i,1]) * xh[i, h+1, w]  + bias[o]

where xv = vertical 3-tap sum of zero-padded x, xh = horizontal 3-tap sum.
(The middle xh row term is absorbed into the xv terms: sum_s xv(shift s) =
full 3x3 box = sum_r xh(shift r).)  5/9 of the PE work of direct conv.

Sharding: data-parallel over batch; 4 images per core on 8 cores.
Output is stored bf16 on device and upcast to fp32 on host (halves the
output DMA traffic; adds <0.4% relative error, tolerance is 2e-2).
"""

import numpy as np
import ml_dtypes

import concourse.bass as bass
import concourse.bacc as bacc
import concourse.mybir as mybir
import concourse.tile as tile
from concourse.bass_utils import run_bass_kernel_spmd

B, CI, CO, H, W = 32, 128, 256, 56, 56
NCORES = 8
BPC = B // NCORES          # images per core
HP, WP = H + 2, W + 2      # padded spatial dims
NPIX = H * W               # 3136
ROWT = 8                   # output rows per matmul tile
NT = H // ROWT             # 7 pixel tiles per image
NTILE = ROWT * W           # 448 columns per matmul
NTERM = 5                  # basis filters (rank of w_eff's spatial subspace)
WARMN = 224                # warmup matmul free dim
WARMUP = 20                # number of warmup matmuls: ~3.7us of PE busy
                           # (one full HAM activity window) so the clock-gate
                           # opens to 2.4 GHz around when the real matmul
                           # stream begins; deliberately short -- a small idle
                           # gap before the real stream is harmless, while
                           # overshooting delays every real matmul

F32 = mybir.dt.float32
BF16 = mybir.dt.bfloat16


def build_nc():
    nc = bacc.Bacc(None, enable_partition_id=False)
    xin = nc.declare_dram_parameter("xp", [BPC, CI, HP, WP], BF16, isOutput=False)
    wk = nc.declare_dram_parameter("klhs", [CI, NTERM, CO], BF16, isOutput=False)
    bb = nc.declare_dram_parameter("bias2", [CI, 2], F32, isOutput=False)
    y = nc.declare_dram_parameter("y", [BPC, CO, H, W], BF16, isOutput=True)

    xflat = xin.rearrange("b c h w -> b c (h w)")
    yflat = y.rearrange("b o h w -> b o (h w)")
    NPAD = HP * WP           # 3364
    NV = H * WP              # 3248 (rows 0..55 of padded, all 58 cols)

    with tile.TileContext(nc) as tc:
        with (
            tc.tile_pool(name="const", bufs=1) as cpool,
            tc.tile_pool(name="xp", bufs=2) as xpool,
            tc.tile_pool(name="xv", bufs=2) as vpool,
            tc.tile_pool(name="xh", bufs=2) as hpool,
            tc.tile_pool(name="yo", bufs=3) as ypool,
            tc.tile_pool(name="ps", bufs=7, space="PSUM") as pspool,
        ):
            klhs_sb = cpool.tile([CI, NTERM * CO], BF16)
            bias_sb = cpool.tile([CI, 2], F32)
            kl3 = klhs_sb.rearrange("i (t o) -> i t o", o=CO)

            # PE warmup: a burst of small matmuls on zeros, issued while the
            # first input DMAs are in flight, trips the HAM clock-gate to
            # 2.4 GHz (~3.4us of sustained PE busy) before the real matmul
            # stream begins, so real matmuls never run at the 1.2 GHz cold
            # rate.
            warm = cpool.tile([128, WARMN], BF16)
            nc.vector.memset(warm[:], 0.0)
            warm_ps = pspool.tile([128, WARMN], F32, bufs=1, tag="warm")
            for _ in range(WARMUP):
                nc.tensor.matmul(
                    warm_ps[:], warm[:, 0:128], warm[:], start=True, stop=True
                )
            warm_out = cpool.tile([128, 32], F32)
            nc.scalar.activation(
                warm_out[:], warm_ps[:, 0:32], mybir.ActivationFunctionType.Copy
            )

            for b in range(BPC):
                # Image 0 gates the whole pipeline: load it in row-chunks so
                # box-sums (and then matmuls) start as soon as possible.
                row_chunks = [12, 32, HP] if b == 0 else [HP]

                xp_sb = xpool.tile([CI, NPAD], BF16)
                xp3d = xflat[b].rearrange("i (h w) -> i h w", w=WP)
                xps3 = xp_sb.rearrange("i (h w) -> i h w", w=WP)
                r0 = 0
                for ci, r1 in enumerate(row_chunks):
                    nc.sync.dma_start(out=xps3[:, r0:r1, :], in_=xp3d[:, r0:r1, :])
                    if b == 0 and ci == 1:
                        # weights/bias queued after the second row chunk --
                        # late enough not to delay chunk 2's data, early
                        # enough (~2us) to land before the first real matmul
                        nc.sync.dma_start(
                            out=klhs_sb[:], in_=wk.rearrange("i t o -> i (t o)")
                        )
                        nc.sync.dma_start(out=bias_sb[:], in_=bb[:])
                    r0 = r1

                # box-sums on the vector engine, emitted in row-chunks so
                # they overlap the loads AND so the next image's first tiles
                # unblock before the whole image is summed:
                # xv[j] = xp[j] + xp[j+58] + xp[j+116]   (rows 0..55)
                # xh[j] = xp[j] + xp[j+1] + xp[j+2]      (rows 0..57, garbage
                #                                         at cols 56/57 unused)
                xvt = vpool.tile([CI, NV], BF16)
                xv = vpool.tile([CI, NV], BF16)
                xht = hpool.tile([CI, NPAD], BF16)
                xh = hpool.tile([CI, NPAD], BF16)

                def boxsum_rows(dst_t, dst, hi, off1, off2, r_a, r_z):
                    # dst[j] = xp[j] + xp[j+off1] + xp[j+off2] for rows r_a..r_z
                    a, z = r_a * WP, (r_z - 1) * WP + hi
                    nc.vector.tensor_add(
                        dst_t[:, a:z], xp_sb[:, a:z], xp_sb[:, a + off1 : z + off1]
                    )
                    nc.vector.tensor_add(
                        dst[:, a:z], dst_t[:, a:z], xp_sb[:, a + off2 : z + off2]
                    )

                v0 = h0r = 0
                for ci, r1 in enumerate(row_chunks):
                    last = ci == len(row_chunks) - 1
                    # emit the sums in small row pieces ordered by when the
                    # matmul stream needs them, so each tile unblocks at the
                    # earliest possible moment
                    if b == 0:
                        mids = {
                            0: [(8, 10), (10, 11)],            # tile 0 first
                            1: [(16, 18), (24, 26), (30, 31)], # t1, t2
                            2: [(34, 36), (H, HP)],            # t3, rest
                        }[ci]
                    elif last:
                        mids = [(26, 28), (H, HP)]
                    else:
                        mids = [(r1 - 2, r1 - 1)]
                    for v1, h1 in mids:
                        if v1 > v0:
                            boxsum_rows(xvt, xv, WP, WP, 2 * WP, v0, v1)
                        if h1 > h0r:
                            boxsum_rows(xht, xh, WP - 2, 1, 2, h0r, h1)
                        v0, h0r = v1, h1

                xv3 = xv.rearrange("i (h w) -> i h w", w=WP)   # [128, 56, 58]
                xh3 = xh.rearrange("i (h w) -> i h w", w=WP)   # [128, 58, 58]

                youts = {}
                last_img = b == BPC - 1

                def emit(half, t, b=b, xv3=xv3, xh3=xh3, youts=youts,
                         last_img=last_img):
                    if half not in youts:
                        youts[half] = ypool.tile(
                            [128, NPIX], BF16, name=f"yout_{b}_{half}", tag="yout"
                        )
                    yout = youts[half]
                    h0 = t * ROWT
                    ps = pspool.tile([128, NTILE], F32, name=f"ps_{b}_{half}_{t}", tag="ps")
                    for s in range(3):
                        nc.tensor.matmul(
                            ps[:],
                            kl3[:, s, half * 128 : half * 128 + 128],
                            xv3[:, h0 : h0 + ROWT, s : s + W],
                            start=(s == 0),
                            stop=False,
                        )
                    nc.tensor.matmul(
                        ps[:],
                        kl3[:, 3, half * 128 : half * 128 + 128],
                        xh3[:, h0 : h0 + ROWT, 0:W],
                        start=False,
                        stop=False,
                    )
                    nc.tensor.matmul(
                        ps[:],
                        kl3[:, 4, half * 128 : half * 128 + 128],
                        xh3[:, h0 + 2 : h0 + 2 + ROWT, 0:W],
                        start=False,
                        stop=True,
                    )
                    last_block = last_img and half == 1
                    nc.scalar.activation(
                        yout[:, t * NTILE : (t + 1) * NTILE],
                        ps[:],
                        mybir.ActivationFunctionType.Identity,
                        bias=bias_sb[:, half : half + 1],
                    )
                    if last_block:
                        # final block: per-tile stores so the kernel tail
                        # only waits on small DMAs
                        if t == 3:
                            nc.sync.dma_start(
                                out=yflat[b, half * 128 : half * 128 + 128, 0 : 4 * NTILE],
                                in_=yout[:, 0 : 4 * NTILE],
                            )
                        elif t >= 4:
                            nc.sync.dma_start(
                                out=yflat[
                                    b,
                                    half * 128 : half * 128 + 128,
                                    t * NTILE : (t + 1) * NTILE,
                                ],
                                in_=yout[:, t * NTILE : (t + 1) * NTILE],
                            )
                    elif t == NT - 1:
                        nc.sync.dma_start(
                            out=yflat[b, half * 128 : half * 128 + 128, :],
                            in_=yout[:],
                        )

                if b == 0:
                    # image 0: interleave halves per tile so the PE never
                    # stalls on box-sums of not-yet-DMAed row chunks
                    order = [(h, t) for t in range(NT) for h in range(2)]
                else:
                    order = [(h, t) for h in range(2) for t in range(NT)]
                for half, t in order:
                    emit(half, t)
    nc.finalize()
    return nc


_NC_CACHE = {}


def _get_nc():
    if "nc" not in _NC_CACHE:
        _NC_CACHE["nc"] = build_nc()
    return _NC_CACHE["nc"]


def make_in_maps(x, weight, bias):
    x = np.asarray(x, dtype=np.float32)
    weight = np.asarray(weight, dtype=np.float32)
    bias = np.asarray(bias, dtype=np.float32)

    krow = weight.sum(axis=3)  # [O, I, 3]
    kcol = weight.sum(axis=2)  # [O, I, 3]
    klhs = np.empty((CI, NTERM, CO), np.float32)
    for s in range(3):
        klhs[:, s, :] = (krow[:, :, s] + kcol[:, :, 1]).T
    klhs[:, 3, :] = (kcol[:, :, 0] - kcol[:, :, 1]).T
    klhs[:, 4, :] = (kcol[:, :, 2] - kcol[:, :, 1]).T
    klhs = klhs.astype(ml_dtypes.bfloat16)

    xp = np.zeros((B, CI, HP, WP), np.float32)
    xp[:, :, 1 : H + 1, 1 : W + 1] = x
    xp = xp.astype(ml_dtypes.bfloat16)

    bias2 = np.ascontiguousarray(bias.reshape(2, 128).T)  # [128, 2] f32

    return [
        {"xp": xp[c * BPC : (c + 1) * BPC], "klhs": klhs, "bias2": bias2}
        for c in range(NCORES)
    ]


def run(in_maps, **kwargs):
    nc = _get_nc()
    return run_bass_kernel_spmd(nc, in_maps, list(range(NCORES)), **kwargs)


def kernel(x, weight, bias):
    res = run(make_in_maps(x, weight, bias))
    out = np.concatenate([res.results[c]["y"] for c in range(NCORES)], axis=0)
    return out.astype(np.float32)
